# revision 3
# baseline (speedup 1.0000x reference)
"""CRF Viterbi decode kernel for Trainium2 (8 NeuronCores, data-parallel batch).

Per core (128 sequences, batch on partitions):

  Phase A (overlapped): X arrives host-pretransposed as [D, S, BC] so DMA
    slabs feed PE matmul lhsT directly (no on-chip transpose, no big ACT
    copies). 8-step chunks -> PSUM -> one ACT copy per chunk into e_store.
    Chunks stream front/back interleaved, just-in-time with the scan.

  Scan: 511 fused pairs, TWO custom DVE ops each (VITSTEP_ANT). Stream =
    26 pages x 27 elems: j=0..25 scan max(acc, T[i,j] + in1[j]); j=26 adds
    the emission e[i] (pure-COUNT 4-state FSM, add-e state does not consume
    src1). out[i,25] = raw page max (beta), out[i,26] = max + e (delta / c).
    The whole recurrence chain lives on the DVE; no per-pair add op.
      fwd: in1 = delta_{k}   (col 26 of prev fwd scratch, stride-27 bcast)
      bwd: in1 = c_{bt+1}    (col 26 of prev bwd scratch); c = beta + e
    ACT persists delta->d_store / beta->b_store (col reads, off-chain);
    GPSIMD parks gamma = delta + beta in-place into d_store.

  Tail: onehot = (gamma >= rowthresh), emitted in 16-step sub-chunks as
    soon as fwd/bwd have both covered them; DMA out overlaps the scan.
    V* mode: gamma rowmax is the SAME global best-path score for every t,
    so one threshold per sequence (computed once from gamma_255) replaces
    per-row reduce_max. eps absorbs f32 summation-order noise.
"""

import numpy as np

B, S, D, L = 1024, 512, 128, 26
NCORES = 8
BC = B // NCORES
HALF = S // 2
NP = S - 1
NB = 3          # in0 buffers per direction
NSC = 4         # scratch slots per direction
SCHUNK = 8
NCH = S // SCHUNK
LA = 4          # phase-A chunk lookahead (chunk-pairs)
TCH = 16        # tail sub-chunk (steps)
TAIL_MODE = "safe"   # "vstar" | "safe"  (vstar measured unsafe on real data:
                     # f32 noise +-0.02 overlaps gamma margins; 4.5K flips)
EPS = 0.05

_BUILD_CACHE = {}


def _np_vitstep(in0, in1, c0, c1, c2):
    i0 = np.asarray(in0, np.float32).reshape(in0.shape[0], -1, 27)
    T = i0[..., :26]
    e = i0[..., 26]
    i1 = np.asarray(in1, np.float32).reshape(in1.shape[0], -1, 26)
    s = T + i1
    r = np.maximum.accumulate(s, axis=-1)
    out = np.empty_like(i0)
    out[..., :26] = r
    out[..., 26] = r[..., 25] + e
    return out.reshape(in0.shape)


def register_vitstep():
    from concourse import dve_spec as Dv
    from concourse import dve_ops as DO
    from concourse.dve_spec import Spec, Src0, scan, AluOp
    from concourse.dve_uop import DveOpSpec, AluInp, Trigger

    for op in DO.OPS:
        if op.name == "VITSTEP_ANT":
            return op

    SRC_DONE = Trigger.SRC_TENSOR_DONE
    CNT = Trigger.COUNT
    NONE = Trigger.NONE

    def _lower_vitstep(spec, ver):
        Dv._validate_body(spec, ver)
        spec2 = Dv._hoist_stream_invariant_ops(spec)
        scans = Dv._collect(spec2.body, Dv.Scan)
        p = Dv._build_placement(spec2, scans, Dv.N_STAGES[ver], Dv.N_LANES[ver])
        base_states = list(Dv._build_state_machine(spec2, scans, [], p))
        assert len(base_states) == 2, base_states  # [seed, steady]
        consume = base_states[1].consume
        assert consume == (True, True)
        sc = scans[0]
        init = Dv._scan_init(sc)
        d = p.node_stage[sc]
        reset_ov = {d: Dv._Stage(sc.op, init, sc.expr)}
        adde_ov = {
            0: Dv._Stage(AluOp.BYPASS, Src0),
            d: Dv._Stage(AluOp.ADD, AluInp.CURR_ALU_OUT, Dv.PREV),
        }
        mk = Dv._State
        states = [
            mk(placement=p, consume=consume, overrides=reset_ov,
               trigger=(SRC_DONE, CNT, NONE), next=(0, 1, 0), repeat=1),
            mk(placement=p, consume=consume,
               trigger=(SRC_DONE, CNT, NONE), next=(0, 2, 0), repeat=25),
            mk(placement=p, consume=(True, False), overrides=adde_ov,
               trigger=(SRC_DONE, CNT, NONE), next=(0, 3, 0), repeat=1),
            mk(placement=p, consume=consume, overrides=reset_ov,
               trigger=(SRC_DONE, CNT, NONE), next=(0, 1, 0), repeat=1),
        ]
        out = [Dv._assemble(s) for s in states]
        for u in out:
            u.validate(ver)
        return out

    class VitDveOp(DO.DveOp):
        def compile(self, ver):
            key = (self.name, ver)
            if (r := DO._COMPILE_CACHE.get(key)) is not None:
                return r
            result = DveOpSpec(
                name=self.name,
                opcode=DO.get_dve_sub_opcode(self.name),
                uops=_lower_vitstep(self.spec, ver),
                rd1_en=DO.has_src1(self.spec),
            )
            DO._COMPILE_CACHE[key] = result
            return result

    from concourse.dve_spec import Src1  # noqa: F401  (body uses Src1)
    spec = Spec(body=scan(Dv.AluOp.MAX, Src0 + Dv.Src1), reference=_np_vitstep)
    op = VitDveOp("VITSTEP_ANT", spec, subdim=False, uops_sha={})
    DO.OPS.append(op)
    DO._SUB_OPCODE_FOR_NAME[op.name] = DO._CUSTOM_DVE_ROW_BASE + len(DO.OPS) - 1
    DO.CUSTOM_DVE_SPECS[op.name] = spec
    assert DO._SUB_OPCODE_FOR_NAME[op.name] < 0x20
    return op


def _build(s_len):
    import concourse.bass as bass
    import concourse.bacc as bacc
    import concourse.tile as tile
    import concourse.mybir as mybir

    vit = register_vitstep()
    alu = mybir.AluOpType
    f32 = mybir.dt.float32

    half = s_len // 2
    np_ = s_len - 1
    nch = s_len // SCHUNK

    nc = bacc.Bacc("TRN2", target_bir_lowering=False, debug=False)
    # X host-pretransposed to [D, S, BC]
    Xh = nc.dram_tensor("XT", (D, s_len, BC), f32, kind="ExternalInput")
    Wh = nc.dram_tensor("W", (D, L), f32, kind="ExternalInput")
    Th = nc.dram_tensor("T", (L, L), f32, kind="ExternalInput")
    Oh = nc.dram_tensor("OUT", (BC, s_len, L), f32, kind="ExternalOutput")

    def ap_at(t, offset_elems, dims):
        a = t[:]
        return bass.AP(tensor=a.tensor, offset=a.offset + offset_elems,
                       ap=[list(a.ap[0])] + dims)

    # phase-A chunk order: front/back interleave
    chunk_order = []
    for c in range(nch // 2):
        chunk_order += [c, nch - 1 - c]
    if nch % 2:
        chunk_order.append(nch // 2)

    # tail sub-chunk schedule: emit_at[pair] -> [subchunk c0 list]
    ntc = s_len // TCH
    tail_at = {}
    post_tail = []
    for c in range(ntc):
        t_lo, t_hi = TCH * c, TCH * c + TCH - 1
        if t_hi < half:
            r = np_ - 1 - t_lo          # last gamma (t_lo) parked at pair np-1-t_lo
        else:
            r = min(t_hi - 1, np_ - 1)  # fwd side; t=s_len-1 parked at np-1
        if TAIL_MODE == "vstar":
            r = max(r, half + 2)
        if r < np_ - 1:
            tail_at.setdefault(r, []).append(c)
        else:
            post_tail.append(c)

    with tile.TileContext(nc) as tc:
        with (
            tc.tile_pool(name="singles", bufs=1) as singles,
            tc.tile_pool(name="xstage", bufs=3) as xstage_p,
            tc.tile_pool(name="ps_e", bufs=4, space="PSUM") as ps_e,
        ):
            e_store = singles.tile([BC, s_len * L], f32)
            d_store = singles.tile([BC, s_len * L], f32)
            b_store = singles.tile([BC, half * L], f32)
            t_cat = singles.tile([BC, 2, L, L], f32)
            in0f = singles.tile([BC, NB, L, 27], f32)
            in0b = singles.tile([BC, NB, L, 27], f32)
            scrf = singles.tile([BC, NSC, 704], f32)
            scrb = singles.tile([BC, NSC, 704], f32)
            gm = singles.tile([BC, TCH], f32)
            vth = singles.tile([BC, 1], f32)
            w_sb = singles.tile([D, L], f32)

            nc.sync.dma_start(w_sb[:], Wh[:])
            t_ap = Th[:]
            nc.sync.dma_start(
                t_cat[:, 1, :, :].rearrange("p a b -> p (a b)"),
                bass.AP(tensor=t_ap.tensor, offset=t_ap.offset,
                        ap=[[0, BC], [1, L * L]]),
            )
            t_flat = singles.tile([1, L * L], f32)
            nc.sync.dma_start(
                t_flat[:],
                bass.AP(tensor=t_ap.tensor, offset=t_ap.offset,
                        ap=[[0, 1], [1, L], [L, L]]),
            )
            nc.gpsimd.partition_broadcast(
                t_cat[:, 0, :, :].rearrange("p a b -> p (a b)"), t_flat[:])

            for b in range(NB):
                nc.vector.tensor_copy(
                    ap_at(in0f, b * (L * 27), [[27, L], [1, L]]), t_cat[:, 0])
                nc.vector.tensor_copy(
                    ap_at(in0b, b * (L * 27), [[27, L], [1, L]]), t_cat[:, 1])

            e3 = e_store.rearrange("p (s l) -> p s l", l=L)
            d3 = d_store.rearrange("p (s l) -> p s l", l=L)
            b3 = b_store.rearrange("p (s l) -> p s l", l=L)

            # ---- phase A emitter (one 8-step chunk) ----
            def emit_chunk(cidx):
                c0 = cidx * SCHUNK
                xs = xstage_p.tile([D, SCHUNK, BC], f32)
                nc.sync.dma_start(xs[:], Xh[:, c0:c0 + SCHUNK, :])
                ep = ps_e.tile([BC, SCHUNK, L], f32)
                for si in range(SCHUNK):
                    nc.tensor.matmul(ep[:, si, :], lhsT=xs[:, si, :],
                                     rhs=w_sb[:], start=True, stop=True)
                nc.scalar.copy(e3[:, c0:c0 + SCHUNK, :], ep[:])

            emitted = 0

            def ensure_chunks(n):
                nonlocal emitted
                while emitted < min(n, nch):
                    emit_chunk(chunk_order[emitted])
                    emitted += 1

            ensure_chunks(2 * LA)

            # init: d3[0] = e_0
            nc.scalar.copy(d3[:, 0, :], e3[:, 0, :])

            # ---- tail emitters ----
            def emit_tail(c):
                c0 = c * TCH
                gsrc = d3[:, c0:c0 + TCH, :]
                if TAIL_MODE == "vstar":
                    nc.vector.tensor_scalar(
                        out=gsrc, in0=gsrc, scalar1=vth[:], scalar2=None,
                        op0=alu.is_ge)
                else:
                    nc.vector.reduce_max(gm[:], gsrc, axis=mybir.AxisListType.X)
                    gm_bc = (gm[:].rearrange("p (t o) -> p t o", o=1)
                             .broadcast_to((BC, TCH, L)))
                    nc.vector.tensor_tensor(gsrc, gsrc, gm_bc, op=alu.is_ge)
                nc.sync.dma_start(
                    Oh[:, c0:c0 + TCH, :].rearrange("p s l -> p (s l)"),
                    d_store[:, c0 * L:(c0 + TCH) * L])

            # ---- scan pairs ----
            for k in range(np_):
                ft, bt = k + 1, s_len - 2 - k
                bf, sf = k % NB, k % NSC
                if k % SCHUNK == 0:
                    ensure_chunks(2 * (k // SCHUNK + LA))
                # ACT: e-slot scatters
                nc.scalar.copy(ap_at(in0f, bf * (L * 27) + 26, [[27, L]]),
                               e3[:, ft, :])
                nc.scalar.copy(ap_at(in0b, bf * (L * 27) + 26, [[27, L]]),
                               e3[:, bt, :])
                # DVE: fused scan steps
                if k == 0:
                    in1f = ap_at(e_store, 0, [[0, L], [1, L]])
                    in1b = ap_at(e_store, (s_len - 1) * L, [[0, L], [1, L]])
                else:
                    pf = ((k - 1) % NSC) * 704 + 26
                    in1f = ap_at(scrf, pf, [[0, L], [27, L]])
                    in1b = ap_at(scrb, pf, [[0, L], [27, L]])
                nc.vector._custom_dve(
                    vit, out=ap_at(scrf, sf * 704, [[27, L], [1, 27]]),
                    in0=ap_at(in0f, bf * (L * 27), [[27, L], [1, 27]]), in1=in1f)
                nc.vector._custom_dve(
                    vit, out=ap_at(scrb, sf * 704, [[27, L], [1, 27]]),
                    in0=ap_at(in0b, bf * (L * 27), [[27, L], [1, 27]]), in1=in1b)
                # ACT: persists
                nc.scalar.copy(d3[:, ft, :], ap_at(scrf, sf * 704 + 26, [[27, L]]))
                if bt >= half:
                    nc.scalar.copy(b3[:, bt - half, :],
                                   ap_at(scrb, sf * 704 + 25, [[27, L]]))
                # GPSIMD: gamma parking (in-place into d3)
                if bt < half:
                    nc.gpsimd.tensor_tensor(
                        d3[:, bt, :], d3[:, bt, :],
                        ap_at(scrb, sf * 704 + 25, [[27, L]]), op=alu.add)
                if k >= half - 1 and ft <= s_len - 2:
                    nc.gpsimd.tensor_tensor(
                        d3[:, ft, :], d3[:, ft, :], b3[:, ft - half, :],
                        op=alu.add)
                # V* threshold once gamma_{half-1} is parked (pair np-half)
                if TAIL_MODE == "vstar" and k == half + 1:
                    nc.vector.reduce_max(vth[:], d3[:, half - 1, :],
                                         axis=mybir.AxisListType.X)
                    nc.vector.tensor_scalar(out=vth[:], in0=vth[:],
                                            scalar1=float(EPS), scalar2=None,
                                            op0=alu.subtract)
                for c in tail_at.get(k, ()):
                    emit_tail(c)

            for c in post_tail:
                emit_tail(c)

    nc.compile()
    return nc


def _get(s_len):
    if s_len not in _BUILD_CACHE:
        _BUILD_CACHE[s_len] = _build(s_len)
    return _BUILD_CACHE[s_len]


LAST_RESULT = None


def kernel(X, W, T):
    global LAST_RESULT
    from concourse.bass_utils import run_bass_kernel_spmd

    X = np.ascontiguousarray(X, dtype=np.float32)
    W = np.ascontiguousarray(W, dtype=np.float32)
    T = np.ascontiguousarray(T, dtype=np.float32)
    s_len = X.shape[1]
    nc = _get(s_len)
    in_maps = []
    for c in range(NCORES):
        xt = np.ascontiguousarray(
            X[c * BC:(c + 1) * BC].transpose(2, 1, 0))  # [D, S, BC]
        in_maps.append({"XT": xt, "W": W, "T": T})
    res = run_bass_kernel_spmd(nc, in_maps, core_ids=list(range(NCORES)))
    LAST_RESULT = res
    return np.concatenate([r["OUT"] for r in res.results], axis=0)


# revision 6
# speedup vs baseline: 1.4682x; 1.4682x over previous
"""CRF Viterbi decode kernel for Trainium2 (8 NeuronCores, data-parallel batch).

Per core (128 sequences, batch on partitions):

  Phase A (overlapped): X arrives host-pretransposed [D, S, BC]; DMA slabs
    feed PE matmul lhsT directly. 4-step chunks -> PSUM -> one ACT copy per
    chunk into e_store, streamed front/back interleaved ahead of the scan.

  Scan: 511 pairs, TWO custom DVE ops each (VITSTEP_ANT, pure-COUNT 4-state
    FSM; stream = 26 pages x 27 elems; j<26: acc = max(acc, T+in1[j]);
    j=26 adds the emission e[page] without consuming src1). Everything is
    kept CONTIGUOUS on the DVE (measured: strided src/dst costs ~1.8x):
    in0 = T+e buffer (contiguous), out = page-major scratch (contiguous),
    in1 = stride-1 broadcast of the previous step's 26 values:
      fwd: in1 = d3[k]      (delta, ACT-persisted from scratch col 26)
      bwd: in1 = c_pp[k-1]  (c = beta+e, ACT-persisted from col 26)
    Each persist runs inside the OTHER direction's op window, so the DVE
    chain never stalls. Interleaved independent ops measure 810 ns each.
    ACT ops are ~390 ns, so the rest is batched: e-slot scatters 2 pairs
    per op (block-reversed for bwd so strides stay positive), beta
    persists 4 pairs per op (negative-stride out). GPSIMD parks
    gamma = delta + beta into d_store in 4-step blocks.
    beta storage: t >= 256 in b3h (half store), t < 256 in a 16-slot
    rolling window (consumed within ~4 pairs by gamma parking).

  Tail: onehot = (gamma >= rowmax) via DVE reduce_max + is_ge in-place in
    d_store, in 16-step sub-chunks as soon as both directions have covered
    them; DMA out overlaps the scan. (A global V*-threshold compare was
    measured unsafe: f32 noise +-0.02 overlaps the margin distribution.)
"""

import numpy as np

B, S, D, L = 1024, 512, 128, 26
NCORES = 8
BC = B // NCORES
HALF = S // 2
NP = S - 1
NSC = 4         # scratch slots per direction
NB2 = 2         # in0 pair-slot buffers per direction (2 pairs each)
SCHUNK = 4
LA = 6          # phase-A lookahead (chunk-pairs)
TCH = 16        # tail sub-chunk (steps)
WB = 16         # b_win slots

_BUILD_CACHE = {}


def _np_vitstep(in0, in1, c0, c1, c2):
    i0 = np.asarray(in0, np.float32).reshape(in0.shape[0], -1, 27)
    T = i0[..., :26]
    e = i0[..., 26]
    i1 = np.asarray(in1, np.float32).reshape(in1.shape[0], -1, 26)[..., :26]
    s = T + i1
    r = np.maximum.accumulate(s, axis=-1)
    out = np.empty_like(i0)
    out[..., :26] = r
    out[..., 26] = r[..., 25] + e
    return out.reshape(in0.shape)


def register_vitstep():
    from concourse import dve_spec as Dv
    from concourse import dve_ops as DO
    from concourse.dve_spec import Spec, Src0, Src1, scan, AluOp
    from concourse.dve_uop import DveOpSpec, AluInp, Trigger

    for op in DO.OPS:
        if op.name == "VITSTEP_ANT":
            return op

    SRC_DONE = Trigger.SRC_TENSOR_DONE
    CNT = Trigger.COUNT
    NONE = Trigger.NONE

    def _lower_vitstep(spec, ver):
        Dv._validate_body(spec, ver)
        spec2 = Dv._hoist_stream_invariant_ops(spec)
        scans = Dv._collect(spec2.body, Dv.Scan)
        p = Dv._build_placement(spec2, scans, Dv.N_STAGES[ver], Dv.N_LANES[ver])
        base_states = list(Dv._build_state_machine(spec2, scans, [], p))
        assert len(base_states) == 2, base_states
        consume = base_states[1].consume
        assert consume == (True, True)
        sc = scans[0]
        init = Dv._scan_init(sc)
        d = p.node_stage[sc]
        reset_ov = {d: Dv._Stage(sc.op, init, sc.expr)}
        adde_ov = {
            0: Dv._Stage(AluOp.BYPASS, Src0),
            d: Dv._Stage(AluOp.ADD, AluInp.CURR_ALU_OUT, Dv.PREV),
        }
        mk = Dv._State
        states = [
            mk(placement=p, consume=consume, overrides=reset_ov,
               trigger=(SRC_DONE, CNT, NONE), next=(0, 1, 0), repeat=1),
            mk(placement=p, consume=consume,
               trigger=(SRC_DONE, CNT, NONE), next=(0, 2, 0), repeat=25),
            mk(placement=p, consume=(True, False), overrides=adde_ov,
               trigger=(SRC_DONE, CNT, NONE), next=(0, 3, 0), repeat=1),
            mk(placement=p, consume=consume, overrides=reset_ov,
               trigger=(SRC_DONE, CNT, NONE), next=(0, 1, 0), repeat=1),
        ]
        out = [Dv._assemble(s) for s in states]
        for u in out:
            u.validate(ver)
        return out

    class VitDveOp(DO.DveOp):
        def compile(self, ver):
            key = (self.name, ver)
            if (r := DO._COMPILE_CACHE.get(key)) is not None:
                return r
            result = DveOpSpec(
                name=self.name,
                opcode=DO.get_dve_sub_opcode(self.name),
                uops=_lower_vitstep(self.spec, ver),
                rd1_en=DO.has_src1(self.spec),
            )
            DO._COMPILE_CACHE[key] = result
            return result

    spec = Spec(body=scan(AluOp.MAX, Src0 + Src1), reference=_np_vitstep)
    op = VitDveOp("VITSTEP_ANT", spec, subdim=False, uops_sha={})
    DO.OPS.append(op)
    DO._SUB_OPCODE_FOR_NAME[op.name] = DO._CUSTOM_DVE_ROW_BASE + len(DO.OPS) - 1
    DO.CUSTOM_DVE_SPECS[op.name] = spec
    assert DO._SUB_OPCODE_FOR_NAME[op.name] < 0x20
    return op


def _build(s_len):
    import concourse.bass as bass
    import concourse.bacc as bacc
    import concourse.tile as tile
    import concourse.mybir as mybir

    vit = register_vitstep()
    alu = mybir.AluOpType
    f32 = mybir.dt.float32

    half = s_len // 2
    np_ = s_len - 1
    nch = s_len // SCHUNK

    nc = bacc.Bacc("TRN2", target_bir_lowering=False, debug=False)
    Xh = nc.dram_tensor("XT", (D, s_len, BC), f32, kind="ExternalInput")
    Wh = nc.dram_tensor("W", (D, L), f32, kind="ExternalInput")
    Th = nc.dram_tensor("T", (L, L), f32, kind="ExternalInput")
    Oh = nc.dram_tensor("OUT", (BC, s_len, L), f32, kind="ExternalOutput")

    def ap_at(t, offset_elems, dims):
        a = t[:]
        return bass.AP(tensor=a.tensor, offset=a.offset + offset_elems,
                       ap=[list(a.ap[0])] + dims)

    chunk_order = []
    for c in range(nch // 2):
        chunk_order += [c, nch - 1 - c]
    if nch % 2:
        chunk_order.append(nch // 2)

    # gamma block schedule: pair -> list of (t0, t1, src) to park
    #   src: ("b3h", off) | ("bwin", slot)
    gamma_at = {}
    for k in range(259, np_ - 3, 4):          # fwd side: ts k-3..k  (256..507)
        gamma_at.setdefault(k, []).append((k - 3, k + 1, "b3h"))
    for t0 in range(4, half, 4):              # bwd side: ts t0..t0+3 (4..255)
        gamma_at.setdefault(514 - t0, []).append((t0, t0 + 4, "bwin"))
    post_gamma = [(0, 4, "bwin"), (s_len - 4, s_len - 1, "b3h")]

    # tail sub-chunk schedule
    ntc = s_len // TCH
    tail_at = {}
    post_tail = []
    for c in range(ntc):
        t_lo, t_hi = TCH * c, TCH * c + TCH - 1
        if t_lo < 4 or t_hi >= s_len - 4:
            post_tail.append(c)
            continue
        r = (515 - t_lo) if t_hi < half else (t_hi + 4)
        if r <= np_ - 2:
            tail_at.setdefault(r, []).append(c)
        else:
            post_tail.append(c)

    with tile.TileContext(nc) as tc:
        with (
            tc.tile_pool(name="singles", bufs=1) as singles,
            tc.tile_pool(name="xstage", bufs=2) as xstage_p,
            tc.tile_pool(name="ps_e", bufs=4, space="PSUM") as ps_e,
        ):
            e_store = singles.tile([BC, s_len * L], f32)
            d_store = singles.tile([BC, s_len * L], f32)
            b3h = singles.tile([BC, half * L], f32)     # beta, t >= half
            b_win = singles.tile([BC, WB * L], f32)     # beta, t < half
            t_cat = singles.tile([BC, 2, L, L], f32)
            # in0 pair-slot bufs: [dir-buf][2 pairs][26 pages][27]
            in0f = singles.tile([BC, NB2, 2, L * 27], f32)
            in0b = singles.tile([BC, NB2, 2, L * 27], f32)
            scrf = singles.tile([BC, NSC, 704], f32)
            scrb = singles.tile([BC, NSC, 704], f32)
            c_pp = singles.tile([BC, 2, L], f32)
            gm = singles.tile([BC, TCH], f32)
            w_sb = singles.tile([D, L], f32)

            nc.sync.dma_start(w_sb[:], Wh[:])
            t_ap = Th[:]
            nc.sync.dma_start(
                t_cat[:, 1, :, :].rearrange("p a b -> p (a b)"),
                bass.AP(tensor=t_ap.tensor, offset=t_ap.offset,
                        ap=[[0, BC], [1, L * L]]),
            )
            t_flat = singles.tile([1, L * L], f32)
            nc.sync.dma_start(
                t_flat[:],
                bass.AP(tensor=t_ap.tensor, offset=t_ap.offset,
                        ap=[[0, 1], [1, L], [L, L]]),
            )
            nc.gpsimd.partition_broadcast(
                t_cat[:, 0, :, :].rearrange("p a b -> p (a b)"), t_flat[:])

            for b in range(NB2):
                for blk in range(2):
                    off = (b * 2 + blk) * (L * 27)
                    nc.vector.tensor_copy(
                        ap_at(in0f, off, [[27, L], [1, L]]), t_cat[:, 0])
                    nc.vector.tensor_copy(
                        ap_at(in0b, off, [[27, L], [1, L]]), t_cat[:, 1])

            e3 = e_store.rearrange("p (s l) -> p s l", l=L)
            d3 = d_store.rearrange("p (s l) -> p s l", l=L)

            def emit_chunk(cidx):
                c0 = cidx * SCHUNK
                xs = xstage_p.tile([D, SCHUNK, BC], f32)
                nc.sync.dma_start(xs[:], Xh[:, c0:c0 + SCHUNK, :])
                ep = ps_e.tile([BC, SCHUNK, L], f32)
                for si in range(SCHUNK):
                    nc.tensor.matmul(ep[:, si, :], lhsT=xs[:, si, :],
                                     rhs=w_sb[:], start=True, stop=True)
                nc.scalar.copy(e3[:, c0:c0 + SCHUNK, :], ep[:])

            emitted = 0

            def ensure_chunks(n):
                nonlocal emitted
                while emitted < min(n, nch):
                    emit_chunk(chunk_order[emitted])
                    emitted += 1

            ensure_chunks(2 * LA)

            # scatter e-slots for the pair group {p, p+1} (p even)
            def scatter_group(p):
                buf = (p // 2) % NB2
                if p == np_ - 1:  # last group has a single pair (510)
                    nc.scalar.copy(
                        ap_at(in0f, (buf * 2) * (L * 27) + 26, [[27, L]]),
                        e3[:, p + 1, :])
                    nc.scalar.copy(
                        ap_at(in0b, (buf * 2 + 1) * (L * 27) + 26, [[27, L]]),
                        e3[:, np_ - 1 - p, :])
                    return
                # fwd: block j holds pair p+j -> e_{p+1+j}; ascending
                nc.scalar.copy(
                    ap_at(in0f, (buf * 2) * (L * 27) + 26,
                          [[L * 27, 2], [27, L]]),
                    e3[:, p + 1:p + 3, :])
                # bwd: block j holds pair p+1-j -> block0=e_{509-p},
                # block1=e_{510-p}; in ascending {509-p, 510-p}
                nc.scalar.copy(
                    ap_at(in0b, (buf * 2) * (L * 27) + 26,
                          [[L * 27, 2], [27, L]]),
                    e3[:, np_ - 2 - p:np_ - p, :])

            # boot: d3[0] = e_0; scatter pair groups {0,1} and {2,3}
            nc.scalar.copy(d3[:, 0, :], e3[:, 0, :])
            scatter_group(0)
            scatter_group(2)

            def emit_tail(c):
                c0 = c * TCH
                gsrc = d3[:, c0:c0 + TCH, :]
                nc.vector.reduce_max(gm[:], gsrc, axis=mybir.AxisListType.X)
                gm_bc = (gm[:].rearrange("p (t o) -> p t o", o=1)
                         .broadcast_to((BC, TCH, L)))
                nc.vector.tensor_tensor(gsrc, gsrc, gm_bc, op=alu.is_ge)
                nc.sync.dma_start(
                    Oh[:, c0:c0 + TCH, :].rearrange("p s l -> p (s l)"),
                    d_store[:, c0 * L:(c0 + TCH) * L])

            def bpersist_to(bt, n, sf0):
                """ACT: b[bt], b[bt-1], .., b[bt-n+1] <- scrb slots sf0..sf0+n-1
                col 25 (bt descending as slot ascends)."""
                src = ap_at(scrb, sf0 * 704 + 25, [[704, n], [27, L]]) if n > 1 \
                    else ap_at(scrb, sf0 * 704 + 25, [[27, L]])
                if bt >= half:
                    assert bt - n + 1 >= half
                    dst = ap_at(b3h, (bt - half) * L, [[-L, n], [1, L]]) \
                        if n > 1 else ap_at(b3h, (bt - half) * L, [[1, L]])
                    nc.scalar.copy(dst, src)
                else:
                    # b_win slots bt%WB descending; split at window wrap
                    done = 0
                    while done < n:
                        b0 = bt - done
                        run = min(n - done, b0 % WB + 1)
                        s = ap_at(scrb, (sf0 + done) * 704 + 25,
                                  [[704, run], [27, L]]) if run > 1 else \
                            ap_at(scrb, (sf0 + done) * 704 + 25, [[27, L]])
                        dd = ap_at(b_win, (b0 % WB) * L, [[-L, run], [1, L]]) \
                            if run > 1 else ap_at(b_win, (b0 % WB) * L, [[1, L]])
                        nc.scalar.copy(dd, s)
                        done += run

            # ---- scan pairs ----
            for k in range(np_):
                ft, bt = k + 1, s_len - 2 - k
                buf, blk, sf = (k // 2) % NB2, k % 2, k % NSC
                if k % SCHUNK == 0:
                    ensure_chunks(2 * (k // SCHUNK + LA))
                # DVE ops
                if k == 0:
                    in1f = ap_at(e_store, 0, [[0, L], [1, L]])
                    in1b = ap_at(e_store, (s_len - 1) * L, [[0, L], [1, L]])
                else:
                    in1f = ap_at(d_store, k * L, [[0, L], [1, L]])
                    in1b = ap_at(c_pp, ((k - 1) % 2) * L, [[0, L], [1, L]])
                nc.vector._custom_dve(
                    vit, out=ap_at(scrf, sf * 704, [[27, L], [1, 27]]),
                    in0=ap_at(in0f, (buf * 2 + blk) * (L * 27),
                              [[27, L], [1, 27]]),
                    in1=in1f)
                nc.vector._custom_dve(
                    vit, out=ap_at(scrb, sf * 704, [[27, L], [1, 27]]),
                    in0=ap_at(in0b, (buf * 2 + (1 - blk)) * (L * 27),
                              [[27, L], [1, 27]]),
                    in1=in1b)
                # ACT chain persists (each hidden under the other dir's op)
                nc.scalar.copy(d3[:, ft, :], ap_at(scrf, sf * 704 + 26,
                                                   [[27, L]]))
                nc.scalar.copy(ap_at(c_pp, (k % 2) * L, [[1, L]]),
                               ap_at(scrb, sf * 704 + 26, [[27, L]]))
                # batched beta persist: at k%4==3, pairs k-3..k (slots 0..3)
                if k % 4 == 3:
                    if bt + 3 >= half and bt < half:
                        nh = bt + 4 - half      # in b3h
                        bpersist_to(bt + 3, nh, 0)
                        bpersist_to(bt + 3 - nh, 4 - nh, nh)
                    else:
                        bpersist_to(bt + 3, 4, 0)
                elif k >= np_ - 3:              # tail pairs 508..510: singles
                    bpersist_to(bt, 1, sf)
                # scatters for pair group {k+3, k+4} at odd k
                if k % 2 == 1 and k + 3 <= np_ - 1:
                    scatter_group(k + 3)
                # GPSIMD gamma parking
                for (t0, t1, src) in gamma_at.get(k, ()):
                    if src == "b3h":
                        bsrc = ap_at(b3h, (t0 - half) * L, [[1, (t1 - t0) * L]])
                    else:
                        bsrc = ap_at(b_win, (t0 % WB) * L, [[1, (t1 - t0) * L]])
                    nc.gpsimd.tensor_tensor(
                        d_store[:, t0 * L:t1 * L],
                        d_store[:, t0 * L:t1 * L], bsrc, op=alu.add)
                for c in tail_at.get(k, ()):
                    emit_tail(c)

            for (t0, t1, src) in post_gamma:
                if src == "b3h":
                    bsrc = ap_at(b3h, (t0 - half) * L, [[1, (t1 - t0) * L]])
                else:
                    bsrc = ap_at(b_win, (t0 % WB) * L, [[1, (t1 - t0) * L]])
                nc.gpsimd.tensor_tensor(
                    d_store[:, t0 * L:t1 * L],
                    d_store[:, t0 * L:t1 * L], bsrc, op=alu.add)
            for c in post_tail:
                emit_tail(c)

    nc.compile()
    return nc


def _get(s_len):
    if s_len not in _BUILD_CACHE:
        _BUILD_CACHE[s_len] = _build(s_len)
    return _BUILD_CACHE[s_len]


LAST_RESULT = None


def kernel(X, W, T):
    global LAST_RESULT
    from concourse.bass_utils import run_bass_kernel_spmd

    X = np.ascontiguousarray(X, dtype=np.float32)
    W = np.ascontiguousarray(W, dtype=np.float32)
    T = np.ascontiguousarray(T, dtype=np.float32)
    s_len = X.shape[1]
    nc = _get(s_len)
    in_maps = []
    for c in range(NCORES):
        xt = np.ascontiguousarray(
            X[c * BC:(c + 1) * BC].transpose(2, 1, 0))  # [D, S, BC]
        in_maps.append({"XT": xt, "W": W, "T": T})
    res = run_bass_kernel_spmd(nc, in_maps, core_ids=list(range(NCORES)))
    LAST_RESULT = res
    return np.concatenate([r["OUT"] for r in res.results], axis=0)


# revision 7
# speedup vs baseline: 1.5204x; 1.0355x over previous
"""CRF Viterbi decode kernel for Trainium2 (8 NeuronCores, data-parallel batch).

Per core (128 sequences, batch on partitions):

  Phase A (overlapped): X arrives host-pretransposed [D, S, BC]; DMA slabs
    feed PE matmul lhsT directly. 4-step chunks -> PSUM -> one ACT copy per
    chunk into e_store, streamed front/back interleaved ahead of the scan.

  Scan: 511 pairs, TWO custom DVE ops each (VITSTEP_ANT, pure-COUNT 4-state
    FSM; stream = 26 pages x 27 elems; j<26: acc = max(acc, T+in1[j]);
    j=26 adds the emission e[page] without consuming src1). Everything is
    kept CONTIGUOUS on the DVE (measured: strided src/dst costs ~1.8x):
    in0 = T+e buffer (contiguous), out = page-major scratch (contiguous),
    in1 = stride-1 broadcast of the previous step's 26 values:
      fwd: in1 = d3[k]      (delta, ACT-persisted from scratch col 26)
      bwd: in1 = c_pp[k-1]  (c = beta+e, ACT-persisted from col 26)
    Each persist runs inside the OTHER direction's op window, so the DVE
    chain never stalls. Interleaved independent ops measure 810 ns each.
    ACT ops are ~390 ns, so the rest is batched: e-slot scatters 2 pairs
    per op (block-reversed for bwd so strides stay positive), beta
    persists 4 pairs per op (negative-stride out). GPSIMD parks
    gamma = delta + beta into d_store in 4-step blocks.
    beta storage: t >= 256 in b3h (half store), t < 256 in a 16-slot
    rolling window (consumed within ~4 pairs by gamma parking).

  Tail: onehot = (gamma >= rowmax) via DVE reduce_max + is_ge in-place in
    d_store, in 16-step sub-chunks as soon as both directions have covered
    them; DMA out overlaps the scan. (A global V*-threshold compare was
    measured unsafe: f32 noise +-0.02 overlaps the margin distribution.)
"""

import numpy as np

B, S, D, L = 1024, 512, 128, 26
NCORES = 8
BC = B // NCORES
HALF = S // 2
NP = S - 1
NSC = 4         # scratch slots per direction
NB2 = 2         # in0 pair-slot buffers per direction (2 pairs each)
SCHUNK = 4
LA = 8          # phase-A lookahead (chunk-pairs)
TCH = 16        # tail sub-chunk (steps)
WB = 16         # b_win slots

_BUILD_CACHE = {}


def _np_vitstep(in0, in1, c0, c1, c2):
    i0 = np.asarray(in0, np.float32).reshape(in0.shape[0], -1, 27)
    T = i0[..., :26]
    e = i0[..., 26]
    i1 = np.asarray(in1, np.float32).reshape(in1.shape[0], -1, 26)[..., :26]
    s = T + i1
    r = np.maximum.accumulate(s, axis=-1)
    out = np.empty_like(i0)
    out[..., :26] = r
    out[..., 26] = r[..., 25] + e
    return out.reshape(in0.shape)


def register_vitstep():
    from concourse import dve_spec as Dv
    from concourse import dve_ops as DO
    from concourse.dve_spec import Spec, Src0, Src1, scan, AluOp
    from concourse.dve_uop import DveOpSpec, AluInp, Trigger

    for op in DO.OPS:
        if op.name == "VITSTEP_ANT":
            return op

    SRC_DONE = Trigger.SRC_TENSOR_DONE
    CNT = Trigger.COUNT
    NONE = Trigger.NONE

    def _lower_vitstep(spec, ver):
        Dv._validate_body(spec, ver)
        spec2 = Dv._hoist_stream_invariant_ops(spec)
        scans = Dv._collect(spec2.body, Dv.Scan)
        p = Dv._build_placement(spec2, scans, Dv.N_STAGES[ver], Dv.N_LANES[ver])
        base_states = list(Dv._build_state_machine(spec2, scans, [], p))
        assert len(base_states) == 2, base_states
        consume = base_states[1].consume
        assert consume == (True, True)
        sc = scans[0]
        init = Dv._scan_init(sc)
        d = p.node_stage[sc]
        reset_ov = {d: Dv._Stage(sc.op, init, sc.expr)}
        adde_ov = {
            0: Dv._Stage(AluOp.BYPASS, Src0),
            d: Dv._Stage(AluOp.ADD, AluInp.CURR_ALU_OUT, Dv.PREV),
        }
        mk = Dv._State
        states = [
            mk(placement=p, consume=consume, overrides=reset_ov,
               trigger=(SRC_DONE, CNT, NONE), next=(0, 1, 0), repeat=1),
            mk(placement=p, consume=consume,
               trigger=(SRC_DONE, CNT, NONE), next=(0, 2, 0), repeat=25),
            mk(placement=p, consume=(True, False), overrides=adde_ov,
               trigger=(SRC_DONE, CNT, NONE), next=(0, 3, 0), repeat=1),
            mk(placement=p, consume=consume, overrides=reset_ov,
               trigger=(SRC_DONE, CNT, NONE), next=(0, 1, 0), repeat=1),
        ]
        out = [Dv._assemble(s) for s in states]
        for u in out:
            u.validate(ver)
        return out

    class VitDveOp(DO.DveOp):
        def compile(self, ver):
            key = (self.name, ver)
            if (r := DO._COMPILE_CACHE.get(key)) is not None:
                return r
            result = DveOpSpec(
                name=self.name,
                opcode=DO.get_dve_sub_opcode(self.name),
                uops=_lower_vitstep(self.spec, ver),
                rd1_en=DO.has_src1(self.spec),
            )
            DO._COMPILE_CACHE[key] = result
            return result

    spec = Spec(body=scan(AluOp.MAX, Src0 + Src1), reference=_np_vitstep)
    op = VitDveOp("VITSTEP_ANT", spec, subdim=False, uops_sha={})
    DO.OPS.append(op)
    DO._SUB_OPCODE_FOR_NAME[op.name] = DO._CUSTOM_DVE_ROW_BASE + len(DO.OPS) - 1
    DO.CUSTOM_DVE_SPECS[op.name] = spec
    assert DO._SUB_OPCODE_FOR_NAME[op.name] < 0x20
    return op


def _build(s_len):
    import concourse.bass as bass
    import concourse.bacc as bacc
    import concourse.tile as tile
    import concourse.mybir as mybir

    vit = register_vitstep()
    alu = mybir.AluOpType
    f32 = mybir.dt.float32

    half = s_len // 2
    np_ = s_len - 1
    nch = s_len // SCHUNK

    nc = bacc.Bacc("TRN2", target_bir_lowering=False, debug=False)
    Xh = nc.dram_tensor("XT", (D, s_len, BC), f32, kind="ExternalInput")
    Wh = nc.dram_tensor("W", (D, L), f32, kind="ExternalInput")
    Th = nc.dram_tensor("T", (L, L), f32, kind="ExternalInput")
    Oh = nc.dram_tensor("OUT", (BC, s_len, L), f32, kind="ExternalOutput")

    def ap_at(t, offset_elems, dims):
        a = t[:]
        return bass.AP(tensor=a.tensor, offset=a.offset + offset_elems,
                       ap=[list(a.ap[0])] + dims)

    chunk_order = []
    for c in range(nch // 2):
        chunk_order += [c, nch - 1 - c]
    if nch % 2:
        chunk_order.append(nch // 2)

    # gamma block schedule: pair -> list of (t0, t1, src) to park
    #   src: ("b3h", off) | ("bwin", slot)
    gamma_at = {}
    for k in range(259, np_ - 3, 4):          # fwd side: ts k-3..k  (256..507)
        gamma_at.setdefault(k, []).append((k - 3, k + 1, "b3h"))
    for t0 in range(4, half, 4):              # bwd side: ts t0..t0+3 (4..255)
        gamma_at.setdefault(514 - t0, []).append((t0, t0 + 4, "bwin"))
    post_gamma = [(0, 4, "bwin"), (s_len - 4, s_len - 1, "b3h")]

    # tail sub-chunk schedule
    ntc = s_len // TCH
    tail_at = {}
    post_tail = []
    for c in range(ntc):
        t_lo, t_hi = TCH * c, TCH * c + TCH - 1
        if t_lo < 4 or t_hi >= s_len - 4:
            post_tail.append(c)
            continue
        r = (515 - t_lo) if t_hi < half else (t_hi + 4)
        if r <= np_ - 2:
            tail_at.setdefault(r, []).append(c)
        else:
            post_tail.append(c)

    with tile.TileContext(nc) as tc:
        with (
            tc.tile_pool(name="singles", bufs=1) as singles,
            tc.tile_pool(name="xstage", bufs=4) as xstage_p,
            tc.tile_pool(name="ps_e", bufs=8, space="PSUM") as ps_e,
        ):
            e_store = singles.tile([BC, s_len * L], f32)
            d_store = singles.tile([BC, s_len * L], f32)
            b3h = singles.tile([BC, half * L], f32)     # beta, t >= half
            b_win = singles.tile([BC, WB * L], f32)     # beta, t < half
            t_cat = singles.tile([BC, 2, L, L], f32)
            # in0 pair-slot bufs: [dir-buf][2 pairs][26 pages][27]
            in0f = singles.tile([BC, NB2, 2, L * 27], f32)
            in0b = singles.tile([BC, NB2, 2, L * 27], f32)
            scrf = singles.tile([BC, NSC, 704], f32)
            scrb = singles.tile([BC, NSC, 704], f32)
            c_pp = singles.tile([BC, 2, L], f32)
            gm = singles.tile([BC, TCH], f32)
            w_sb = singles.tile([D, L], f32)

            nc.sync.dma_start(w_sb[:], Wh[:])
            t_ap = Th[:]
            nc.sync.dma_start(
                t_cat[:, 1, :, :].rearrange("p a b -> p (a b)"),
                bass.AP(tensor=t_ap.tensor, offset=t_ap.offset,
                        ap=[[0, BC], [1, L * L]]),
            )
            t_flat = singles.tile([1, L * L], f32)
            nc.sync.dma_start(
                t_flat[:],
                bass.AP(tensor=t_ap.tensor, offset=t_ap.offset,
                        ap=[[0, 1], [1, L], [L, L]]),
            )
            nc.gpsimd.partition_broadcast(
                t_cat[:, 0, :, :].rearrange("p a b -> p (a b)"), t_flat[:])

            for b in range(NB2):
                for blk in range(2):
                    off = (b * 2 + blk) * (L * 27)
                    nc.vector.tensor_copy(
                        ap_at(in0f, off, [[27, L], [1, L]]), t_cat[:, 0])
                    nc.vector.tensor_copy(
                        ap_at(in0b, off, [[27, L], [1, L]]), t_cat[:, 1])

            e3 = e_store.rearrange("p (s l) -> p s l", l=L)
            d3 = d_store.rearrange("p (s l) -> p s l", l=L)

            def emit_chunk(cidx):
                c0 = cidx * SCHUNK
                xs = xstage_p.tile([D, SCHUNK, BC], f32)
                nc.sync.dma_start(xs[:], Xh[:, c0:c0 + SCHUNK, :])
                ep = ps_e.tile([BC, SCHUNK, L], f32)
                for si in range(SCHUNK):
                    nc.tensor.matmul(ep[:, si, :], lhsT=xs[:, si, :],
                                     rhs=w_sb[:], start=True, stop=True)
                nc.scalar.copy(e3[:, c0:c0 + SCHUNK, :], ep[:])

            emitted = 0

            def ensure_chunks(n):
                nonlocal emitted
                while emitted < min(n, nch):
                    emit_chunk(chunk_order[emitted])
                    emitted += 1

            ensure_chunks(2 * LA)

            # scatter e-slots for the pair group {p, p+1} (p even)
            def scatter_group(p):
                buf = (p // 2) % NB2
                if p == np_ - 1:  # last group has a single pair (510)
                    nc.scalar.copy(
                        ap_at(in0f, (buf * 2) * (L * 27) + 26, [[27, L]]),
                        e3[:, p + 1, :])
                    nc.scalar.copy(
                        ap_at(in0b, (buf * 2 + 1) * (L * 27) + 26, [[27, L]]),
                        e3[:, np_ - 1 - p, :])
                    return
                # fwd: block j holds pair p+j -> e_{p+1+j}; ascending
                nc.scalar.copy(
                    ap_at(in0f, (buf * 2) * (L * 27) + 26,
                          [[L * 27, 2], [27, L]]),
                    e3[:, p + 1:p + 3, :])
                # bwd: block j holds pair p+1-j -> block0=e_{509-p},
                # block1=e_{510-p}; in ascending {509-p, 510-p}
                nc.scalar.copy(
                    ap_at(in0b, (buf * 2) * (L * 27) + 26,
                          [[L * 27, 2], [27, L]]),
                    e3[:, np_ - 2 - p:np_ - p, :])

            # boot: d3[0] = e_0; scatter pair groups {0,1} and {2,3}
            nc.scalar.copy(d3[:, 0, :], e3[:, 0, :])
            scatter_group(0)
            scatter_group(2)

            def emit_tail(c):
                c0 = c * TCH
                gsrc = d3[:, c0:c0 + TCH, :]
                nc.vector.reduce_max(gm[:], gsrc, axis=mybir.AxisListType.X)
                gm_bc = (gm[:].rearrange("p (t o) -> p t o", o=1)
                         .broadcast_to((BC, TCH, L)))
                nc.vector.tensor_tensor(gsrc, gsrc, gm_bc, op=alu.is_ge)
                nc.sync.dma_start(
                    Oh[:, c0:c0 + TCH, :].rearrange("p s l -> p (s l)"),
                    d_store[:, c0 * L:(c0 + TCH) * L])

            def bpersist_to(bt, n, sf0):
                """ACT: b[bt], b[bt-1], .., b[bt-n+1] <- scrb slots sf0..sf0+n-1
                col 25 (bt descending as slot ascends)."""
                src = ap_at(scrb, sf0 * 704 + 25, [[704, n], [27, L]]) if n > 1 \
                    else ap_at(scrb, sf0 * 704 + 25, [[27, L]])
                if bt >= half:
                    assert bt - n + 1 >= half
                    dst = ap_at(b3h, (bt - half) * L, [[-L, n], [1, L]]) \
                        if n > 1 else ap_at(b3h, (bt - half) * L, [[1, L]])
                    nc.scalar.copy(dst, src)
                else:
                    # b_win slots bt%WB descending; split at window wrap
                    done = 0
                    while done < n:
                        b0 = bt - done
                        run = min(n - done, b0 % WB + 1)
                        s = ap_at(scrb, (sf0 + done) * 704 + 25,
                                  [[704, run], [27, L]]) if run > 1 else \
                            ap_at(scrb, (sf0 + done) * 704 + 25, [[27, L]])
                        dd = ap_at(b_win, (b0 % WB) * L, [[-L, run], [1, L]]) \
                            if run > 1 else ap_at(b_win, (b0 % WB) * L, [[1, L]])
                        nc.scalar.copy(dd, s)
                        done += run

            # ---- scan pairs ----
            for k in range(np_):
                ft, bt = k + 1, s_len - 2 - k
                buf, blk, sf = (k // 2) % NB2, k % 2, k % NSC
                # DVE ops
                if k == 0:
                    in1f = ap_at(e_store, 0, [[0, L], [1, L]])
                    in1b = ap_at(e_store, (s_len - 1) * L, [[0, L], [1, L]])
                else:
                    in1f = ap_at(d_store, k * L, [[0, L], [1, L]])
                    in1b = ap_at(c_pp, ((k - 1) % 2) * L, [[0, L], [1, L]])
                nc.vector._custom_dve(
                    vit, out=ap_at(scrf, sf * 704, [[27, L], [1, 27]]),
                    in0=ap_at(in0f, (buf * 2 + blk) * (L * 27),
                              [[27, L], [1, 27]]),
                    in1=in1f)
                nc.vector._custom_dve(
                    vit, out=ap_at(scrb, sf * 704, [[27, L], [1, 27]]),
                    in0=ap_at(in0b, (buf * 2 + (1 - blk)) * (L * 27),
                              [[27, L], [1, 27]]),
                    in1=in1b)
                # ACT chain persists (each hidden under the other dir's op)
                nc.scalar.copy(d3[:, ft, :], ap_at(scrf, sf * 704 + 26,
                                                   [[27, L]]))
                nc.scalar.copy(ap_at(c_pp, (k % 2) * L, [[1, L]]),
                               ap_at(scrb, sf * 704 + 26, [[27, L]]))
                # batched beta persist: at k%4==3, pairs k-3..k (slots 0..3)
                if k % 4 == 3:
                    if bt + 3 >= half and bt < half:
                        nh = bt + 4 - half      # in b3h
                        bpersist_to(bt + 3, nh, 0)
                        bpersist_to(bt + 3 - nh, 4 - nh, nh)
                    else:
                        bpersist_to(bt + 3, 4, 0)
                elif k >= np_ - 3:              # tail pairs 508..510: singles
                    bpersist_to(bt, 1, sf)
                # scatters for pair group {k+3, k+4} at odd k
                if k % 2 == 1 and k + 3 <= np_ - 1:
                    scatter_group(k + 3)
                # GPSIMD gamma parking
                for (t0, t1, src) in gamma_at.get(k, ()):
                    if src == "b3h":
                        bsrc = ap_at(b3h, (t0 - half) * L, [[1, (t1 - t0) * L]])
                    else:
                        bsrc = ap_at(b_win, (t0 % WB) * L, [[1, (t1 - t0) * L]])
                    nc.gpsimd.tensor_tensor(
                        d_store[:, t0 * L:t1 * L],
                        d_store[:, t0 * L:t1 * L], bsrc, op=alu.add)
                for c in tail_at.get(k, ()):
                    emit_tail(c)
                if k % 2 == 0:
                    ensure_chunks(k // 2 + 2 * LA)

            for (t0, t1, src) in post_gamma:
                if src == "b3h":
                    bsrc = ap_at(b3h, (t0 - half) * L, [[1, (t1 - t0) * L]])
                else:
                    bsrc = ap_at(b_win, (t0 % WB) * L, [[1, (t1 - t0) * L]])
                nc.gpsimd.tensor_tensor(
                    d_store[:, t0 * L:t1 * L],
                    d_store[:, t0 * L:t1 * L], bsrc, op=alu.add)
            for c in post_tail:
                emit_tail(c)

    nc.compile()
    return nc


def _get(s_len):
    if s_len not in _BUILD_CACHE:
        _BUILD_CACHE[s_len] = _build(s_len)
    return _BUILD_CACHE[s_len]


LAST_RESULT = None


def kernel(X, W, T):
    global LAST_RESULT
    from concourse.bass_utils import run_bass_kernel_spmd

    X = np.ascontiguousarray(X, dtype=np.float32)
    W = np.ascontiguousarray(W, dtype=np.float32)
    T = np.ascontiguousarray(T, dtype=np.float32)
    s_len = X.shape[1]
    nc = _get(s_len)
    in_maps = []
    for c in range(NCORES):
        xt = np.ascontiguousarray(
            X[c * BC:(c + 1) * BC].transpose(2, 1, 0))  # [D, S, BC]
        in_maps.append({"XT": xt, "W": W, "T": T})
    res = run_bass_kernel_spmd(nc, in_maps, core_ids=list(range(NCORES)))
    LAST_RESULT = res
    return np.concatenate([r["OUT"] for r in res.results], axis=0)


# revision 9
# speedup vs baseline: 1.5226x; 1.0015x over previous
"""CRF Viterbi decode kernel for Trainium2 (8 NeuronCores, data-parallel batch).

Per core (128 sequences, batch on partitions):

  Phase A (overlapped): X arrives host-pretransposed [D, S, BC]; DMA slabs
    feed PE matmul lhsT directly. 4-step chunks -> PSUM -> one ACT copy per
    chunk into e_store, streamed front/back interleaved ahead of the scan.

  Scan: 511 pairs, TWO custom DVE ops each (VITSTEP_ANT, pure-COUNT 4-state
    FSM; stream = 26 pages x 27 elems; j<26: acc = max(acc, T+in1[j]);
    j=26 adds the emission e[page] without consuming src1). Everything is
    kept CONTIGUOUS on the DVE (measured: strided src/dst costs ~1.8x):
    in0 = T+e buffer (contiguous), out = page-major scratch (contiguous),
    in1 = stride-1 broadcast of the previous step's 26 values:
      fwd: in1 = d3[k]      (delta, ACT-persisted from scratch col 26)
      bwd: in1 = c_pp[k-1]  (c = beta+e, ACT-persisted from col 26)
    Each persist runs inside the OTHER direction's op window, so the DVE
    chain never stalls. Interleaved independent ops measure 810 ns each.
    ACT ops are ~390 ns, so the rest is batched: e-slot scatters 2 pairs
    per op (block-reversed for bwd so strides stay positive), beta
    persists 4 pairs per op (negative-stride out). GPSIMD parks
    gamma = delta + beta into d_store in 4-step blocks.
    beta storage: t >= 256 in b3h (half store), t < 256 in a 16-slot
    rolling window (consumed within ~4 pairs by gamma parking).

  Tail: onehot = (gamma >= rowmax) via DVE reduce_max + is_ge in-place in
    d_store, in 16-step sub-chunks as soon as both directions have covered
    them; DMA out overlaps the scan. (A global V*-threshold compare was
    measured unsafe: f32 noise +-0.02 overlaps the margin distribution.)
"""

import numpy as np

B, S, D, L = 1024, 512, 128, 26
NCORES = 8
BC = B // NCORES
HALF = S // 2
NP = S - 1
NSC = 4         # scratch slots per direction
NB2 = 2         # in0 pair-slot buffers per direction (2 pairs each)
SCHUNK = 4
LA = 8          # phase-A lookahead (chunk-pairs)
TCH = 32        # tail sub-chunk (steps)
WB = 16         # b_win slots

_BUILD_CACHE = {}


def _np_vitstep(in0, in1, c0, c1, c2):
    i0 = np.asarray(in0, np.float32).reshape(in0.shape[0], -1, 27)
    T = i0[..., :26]
    e = i0[..., 26]
    i1 = np.asarray(in1, np.float32).reshape(in1.shape[0], -1, 26)[..., :26]
    s = T + i1
    r = np.maximum.accumulate(s, axis=-1)
    out = np.empty_like(i0)
    out[..., :26] = r
    out[..., 26] = r[..., 25] + e
    return out.reshape(in0.shape)


def register_vitstep():
    from concourse import dve_spec as Dv
    from concourse import dve_ops as DO
    from concourse.dve_spec import Spec, Src0, Src1, scan, AluOp
    from concourse.dve_uop import DveOpSpec, AluInp, Trigger

    for op in DO.OPS:
        if op.name == "VITSTEP_ANT":
            return op

    SRC_DONE = Trigger.SRC_TENSOR_DONE
    CNT = Trigger.COUNT
    NONE = Trigger.NONE

    def _lower_vitstep(spec, ver):
        Dv._validate_body(spec, ver)
        spec2 = Dv._hoist_stream_invariant_ops(spec)
        scans = Dv._collect(spec2.body, Dv.Scan)
        p = Dv._build_placement(spec2, scans, Dv.N_STAGES[ver], Dv.N_LANES[ver])
        base_states = list(Dv._build_state_machine(spec2, scans, [], p))
        assert len(base_states) == 2, base_states
        consume = base_states[1].consume
        assert consume == (True, True)
        sc = scans[0]
        init = Dv._scan_init(sc)
        d = p.node_stage[sc]
        reset_ov = {d: Dv._Stage(sc.op, init, sc.expr)}
        adde_ov = {
            0: Dv._Stage(AluOp.BYPASS, Src0),
            d: Dv._Stage(AluOp.ADD, AluInp.CURR_ALU_OUT, Dv.PREV),
        }
        mk = Dv._State
        states = [
            mk(placement=p, consume=consume, overrides=reset_ov,
               trigger=(SRC_DONE, CNT, NONE), next=(0, 1, 0), repeat=1),
            mk(placement=p, consume=consume,
               trigger=(SRC_DONE, CNT, NONE), next=(0, 2, 0), repeat=25),
            mk(placement=p, consume=(True, False), overrides=adde_ov,
               trigger=(SRC_DONE, CNT, NONE), next=(0, 3, 0), repeat=1),
            mk(placement=p, consume=consume, overrides=reset_ov,
               trigger=(SRC_DONE, CNT, NONE), next=(0, 1, 0), repeat=1),
        ]
        out = [Dv._assemble(s) for s in states]
        for u in out:
            u.validate(ver)
        return out

    class VitDveOp(DO.DveOp):
        def compile(self, ver):
            key = (self.name, ver)
            if (r := DO._COMPILE_CACHE.get(key)) is not None:
                return r
            result = DveOpSpec(
                name=self.name,
                opcode=DO.get_dve_sub_opcode(self.name),
                uops=_lower_vitstep(self.spec, ver),
                rd1_en=DO.has_src1(self.spec),
            )
            DO._COMPILE_CACHE[key] = result
            return result

    spec = Spec(body=scan(AluOp.MAX, Src0 + Src1), reference=_np_vitstep)
    op = VitDveOp("VITSTEP_ANT", spec, subdim=False, uops_sha={})
    DO.OPS.append(op)
    DO._SUB_OPCODE_FOR_NAME[op.name] = DO._CUSTOM_DVE_ROW_BASE + len(DO.OPS) - 1
    DO.CUSTOM_DVE_SPECS[op.name] = spec
    assert DO._SUB_OPCODE_FOR_NAME[op.name] < 0x20
    return op


def _build(s_len):
    import concourse.bass as bass
    import concourse.bacc as bacc
    import concourse.tile as tile
    import concourse.mybir as mybir

    vit = register_vitstep()
    alu = mybir.AluOpType
    f32 = mybir.dt.float32

    half = s_len // 2
    np_ = s_len - 1
    nch = s_len // SCHUNK

    nc = bacc.Bacc("TRN2", target_bir_lowering=False, debug=False)
    Xh = nc.dram_tensor("XT", (D, s_len, BC), f32, kind="ExternalInput")
    Wh = nc.dram_tensor("W", (D, L), f32, kind="ExternalInput")
    Th = nc.dram_tensor("T", (L, L), f32, kind="ExternalInput")
    Oh = nc.dram_tensor("OUT", (BC, s_len, L), f32, kind="ExternalOutput")

    def ap_at(t, offset_elems, dims):
        a = t[:]
        return bass.AP(tensor=a.tensor, offset=a.offset + offset_elems,
                       ap=[list(a.ap[0])] + dims)

    chunk_order = []
    for c in range(nch // 2):
        chunk_order += [c, nch - 1 - c]
    if nch % 2:
        chunk_order.append(nch // 2)

    # gamma block schedule: pair -> list of (t0, t1, src) to park
    #   src: ("b3h", off) | ("bwin", slot)
    gamma_at = {}
    for k in range(259, np_ - 3, 4):          # fwd side: ts k-3..k  (256..507)
        gamma_at.setdefault(k, []).append((k - 3, k + 1, "b3h"))
    for t0 in range(4, half, 4):              # bwd side: ts t0..t0+3 (4..255)
        gamma_at.setdefault(514 - t0, []).append((t0, t0 + 4, "bwin"))
    post_gamma = [(0, 4, "bwin"), (s_len - 4, s_len - 1, "b3h")]

    # tail sub-chunk schedule
    ntc = s_len // TCH
    tail_at = {}
    post_tail = []
    for c in range(ntc):
        t_lo, t_hi = TCH * c, TCH * c + TCH - 1
        if t_lo < 4 or t_hi >= s_len - 4:
            post_tail.append(c)
            continue
        r = (515 - t_lo) if t_hi < half else (t_hi + 4)
        if r <= np_ - 2:
            tail_at.setdefault(r, []).append(c)
        else:
            post_tail.append(c)

    with tile.TileContext(nc) as tc:
        with (
            tc.tile_pool(name="singles", bufs=1) as singles,
            tc.tile_pool(name="xstage", bufs=4) as xstage_p,
            tc.tile_pool(name="ps_e", bufs=8, space="PSUM") as ps_e,
        ):
            e_store = singles.tile([BC, s_len * L], f32)
            d_store = singles.tile([BC, s_len * L], f32)
            b3h = singles.tile([BC, half * L], f32)     # beta, t >= half
            b_win = singles.tile([BC, WB * L], f32)     # beta, t < half
            t_cat = singles.tile([BC, 2, L, L], f32)
            # in0 pair-slot bufs: [dir-buf][2 pairs][26 pages][27]
            in0f = singles.tile([BC, NB2, 2, L * 27], f32)
            in0b = singles.tile([BC, NB2, 2, L * 27], f32)
            scrf = singles.tile([BC, NSC, 704], f32)
            scrb = singles.tile([BC, NSC, 704], f32)
            c_pp = singles.tile([BC, 2, L], f32)
            gm = singles.tile([BC, TCH], f32)
            w_sb = singles.tile([D, L], f32)
            e3 = e_store.rearrange("p (s l) -> p s l", l=L)

            nc.sync.dma_start(w_sb[:], Wh[:])

            def emit_chunk(cidx):
                c0 = cidx * SCHUNK
                xs = xstage_p.tile([D, SCHUNK, BC], f32)
                nc.sync.dma_start(xs[:], Xh[:, c0:c0 + SCHUNK, :])
                ep = ps_e.tile([BC, SCHUNK, L], f32)
                for si in range(SCHUNK):
                    nc.tensor.matmul(ep[:, si, :], lhsT=xs[:, si, :],
                                     rhs=w_sb[:], start=True, stop=True)
                nc.scalar.copy(e3[:, c0:c0 + SCHUNK, :], ep[:])

            emitted = 0

            def ensure_chunks(n):
                nonlocal emitted
                while emitted < min(n, nch):
                    emit_chunk(chunk_order[emitted])
                    emitted += 1

            ensure_chunks(4)
            t_ap = Th[:]
            nc.sync.dma_start(
                t_cat[:, 1, :, :].rearrange("p a b -> p (a b)"),
                bass.AP(tensor=t_ap.tensor, offset=t_ap.offset,
                        ap=[[0, BC], [1, L * L]]),
            )
            t_flat = singles.tile([1, L * L], f32)
            nc.sync.dma_start(
                t_flat[:],
                bass.AP(tensor=t_ap.tensor, offset=t_ap.offset,
                        ap=[[0, 1], [1, L], [L, L]]),
            )
            nc.gpsimd.partition_broadcast(
                t_cat[:, 0, :, :].rearrange("p a b -> p (a b)"), t_flat[:])

            for b in range(NB2):
                for blk in range(2):
                    off = (b * 2 + blk) * (L * 27)
                    nc.vector.tensor_copy(
                        ap_at(in0f, off, [[27, L], [1, L]]), t_cat[:, 0])
                    nc.vector.tensor_copy(
                        ap_at(in0b, off, [[27, L], [1, L]]), t_cat[:, 1])

            d3 = d_store.rearrange("p (s l) -> p s l", l=L)


            # scatter e-slots for the pair group {p, p+1} (p even)
            def scatter_group(p):
                buf = (p // 2) % NB2
                if p == np_ - 1:  # last group has a single pair (510)
                    nc.scalar.copy(
                        ap_at(in0f, (buf * 2) * (L * 27) + 26, [[27, L]]),
                        e3[:, p + 1, :])
                    nc.scalar.copy(
                        ap_at(in0b, (buf * 2 + 1) * (L * 27) + 26, [[27, L]]),
                        e3[:, np_ - 1 - p, :])
                    return
                # fwd: block j holds pair p+j -> e_{p+1+j}; ascending
                nc.scalar.copy(
                    ap_at(in0f, (buf * 2) * (L * 27) + 26,
                          [[L * 27, 2], [27, L]]),
                    e3[:, p + 1:p + 3, :])
                # bwd: block j holds pair p+1-j -> block0=e_{509-p},
                # block1=e_{510-p}; in ascending {509-p, 510-p}
                nc.scalar.copy(
                    ap_at(in0b, (buf * 2) * (L * 27) + 26,
                          [[L * 27, 2], [27, L]]),
                    e3[:, np_ - 2 - p:np_ - p, :])

            # boot: d3[0] = e_0; scatter pair groups {0,1} and {2,3}
            nc.scalar.copy(d3[:, 0, :], e3[:, 0, :])
            scatter_group(0)
            scatter_group(2)

            def emit_tail(c):
                c0 = c * TCH
                gsrc = d3[:, c0:c0 + TCH, :]
                nc.vector.reduce_max(gm[:], gsrc, axis=mybir.AxisListType.X)
                gm_bc = (gm[:].rearrange("p (t o) -> p t o", o=1)
                         .broadcast_to((BC, TCH, L)))
                nc.vector.tensor_tensor(gsrc, gsrc, gm_bc, op=alu.is_ge)
                nc.sync.dma_start(
                    Oh[:, c0:c0 + TCH, :].rearrange("p s l -> p (s l)"),
                    d_store[:, c0 * L:(c0 + TCH) * L])

            def bpersist_to(bt, n, sf0):
                """ACT: b[bt], b[bt-1], .., b[bt-n+1] <- scrb slots sf0..sf0+n-1
                col 25 (bt descending as slot ascends)."""
                src = ap_at(scrb, sf0 * 704 + 25, [[704, n], [27, L]]) if n > 1 \
                    else ap_at(scrb, sf0 * 704 + 25, [[27, L]])
                if bt >= half:
                    assert bt - n + 1 >= half
                    dst = ap_at(b3h, (bt - half) * L, [[-L, n], [1, L]]) \
                        if n > 1 else ap_at(b3h, (bt - half) * L, [[1, L]])
                    nc.scalar.copy(dst, src)
                else:
                    # b_win slots bt%WB descending; split at window wrap
                    done = 0
                    while done < n:
                        b0 = bt - done
                        run = min(n - done, b0 % WB + 1)
                        s = ap_at(scrb, (sf0 + done) * 704 + 25,
                                  [[704, run], [27, L]]) if run > 1 else \
                            ap_at(scrb, (sf0 + done) * 704 + 25, [[27, L]])
                        dd = ap_at(b_win, (b0 % WB) * L, [[-L, run], [1, L]]) \
                            if run > 1 else ap_at(b_win, (b0 % WB) * L, [[1, L]])
                        nc.scalar.copy(dd, s)
                        done += run

            # ---- scan pairs ----
            for k in range(np_):
                ft, bt = k + 1, s_len - 2 - k
                buf, blk, sf = (k // 2) % NB2, k % 2, k % NSC
                # DVE ops
                if k == 0:
                    in1f = ap_at(e_store, 0, [[0, L], [1, L]])
                    in1b = ap_at(e_store, (s_len - 1) * L, [[0, L], [1, L]])
                else:
                    in1f = ap_at(d_store, k * L, [[0, L], [1, L]])
                    in1b = ap_at(c_pp, ((k - 1) % 2) * L, [[0, L], [1, L]])
                nc.vector._custom_dve(
                    vit, out=ap_at(scrf, sf * 704, [[27, L], [1, 27]]),
                    in0=ap_at(in0f, (buf * 2 + blk) * (L * 27),
                              [[27, L], [1, 27]]),
                    in1=in1f)
                nc.vector._custom_dve(
                    vit, out=ap_at(scrb, sf * 704, [[27, L], [1, 27]]),
                    in0=ap_at(in0b, (buf * 2 + (1 - blk)) * (L * 27),
                              [[27, L], [1, 27]]),
                    in1=in1b)
                # ACT chain persists (each hidden under the other dir's op)
                nc.scalar.copy(d3[:, ft, :], ap_at(scrf, sf * 704 + 26,
                                                   [[27, L]]))
                nc.scalar.copy(ap_at(c_pp, (k % 2) * L, [[1, L]]),
                               ap_at(scrb, sf * 704 + 26, [[27, L]]))
                # batched beta persist: at k%4==3, pairs k-3..k (slots 0..3)
                if k % 4 == 3:
                    if bt + 3 >= half and bt < half:
                        nh = bt + 4 - half      # in b3h
                        bpersist_to(bt + 3, nh, 0)
                        bpersist_to(bt + 3 - nh, 4 - nh, nh)
                    else:
                        bpersist_to(bt + 3, 4, 0)
                elif k >= np_ - 3:              # tail pairs 508..510: singles
                    bpersist_to(bt, 1, sf)
                # scatters for pair group {k+3, k+4} at odd k
                if k % 2 == 1 and k + 3 <= np_ - 1:
                    scatter_group(k + 3)
                # GPSIMD gamma parking
                for (t0, t1, src) in gamma_at.get(k, ()):
                    if src == "b3h":
                        bsrc = ap_at(b3h, (t0 - half) * L, [[1, (t1 - t0) * L]])
                    else:
                        bsrc = ap_at(b_win, (t0 % WB) * L, [[1, (t1 - t0) * L]])
                    nc.gpsimd.tensor_tensor(
                        d_store[:, t0 * L:t1 * L],
                        d_store[:, t0 * L:t1 * L], bsrc, op=alu.add)
                for c in tail_at.get(k, ()):
                    emit_tail(c)
                if k % 2 == 0:
                    ensure_chunks(min(4 + k, k // 2 + 2 * LA))

            for (t0, t1, src) in post_gamma:
                if src == "b3h":
                    bsrc = ap_at(b3h, (t0 - half) * L, [[1, (t1 - t0) * L]])
                else:
                    bsrc = ap_at(b_win, (t0 % WB) * L, [[1, (t1 - t0) * L]])
                nc.gpsimd.tensor_tensor(
                    d_store[:, t0 * L:t1 * L],
                    d_store[:, t0 * L:t1 * L], bsrc, op=alu.add)
            for c in post_tail:
                emit_tail(c)

    nc.compile()
    return nc


def _get(s_len):
    if s_len not in _BUILD_CACHE:
        _BUILD_CACHE[s_len] = _build(s_len)
    return _BUILD_CACHE[s_len]


LAST_RESULT = None


def kernel(X, W, T):
    global LAST_RESULT
    from concourse.bass_utils import run_bass_kernel_spmd

    X = np.ascontiguousarray(X, dtype=np.float32)
    W = np.ascontiguousarray(W, dtype=np.float32)
    T = np.ascontiguousarray(T, dtype=np.float32)
    s_len = X.shape[1]
    nc = _get(s_len)
    in_maps = []
    for c in range(NCORES):
        xt = np.ascontiguousarray(
            X[c * BC:(c + 1) * BC].transpose(2, 1, 0))  # [D, S, BC]
        in_maps.append({"XT": xt, "W": W, "T": T})
    res = run_bass_kernel_spmd(nc, in_maps, core_ids=list(range(NCORES)))
    LAST_RESULT = res
    return np.concatenate([r["OUT"] for r in res.results], axis=0)


# revision 11
# speedup vs baseline: 1.5246x; 1.0013x over previous
"""CRF Viterbi decode kernel for Trainium2 (8 NeuronCores, data-parallel batch).

Per core (128 sequences, batch on partitions):

  Phase A (overlapped): X arrives host-pretransposed [D, S, BC]; DMA slabs
    feed PE matmul lhsT directly. 4-step chunks -> PSUM -> one ACT copy per
    chunk into e_store, streamed front/back interleaved ahead of the scan.

  Scan: 511 pairs, TWO custom DVE ops each (VITSTEP_ANT, pure-COUNT 4-state
    FSM; stream = 26 pages x 27 elems; j<26: acc = max(acc, T+in1[j]);
    j=26 adds the emission e[page] without consuming src1). Everything is
    kept CONTIGUOUS on the DVE (measured: strided src/dst costs ~1.8x):
    in0 = T+e buffer (contiguous), out = page-major scratch (contiguous),
    in1 = stride-1 broadcast of the previous step's 26 values:
      fwd: in1 = d3[k]      (delta, ACT-persisted from scratch col 26)
      bwd: in1 = c_pp[k-1]  (c = beta+e, ACT-persisted from col 26)
    Each persist runs inside the OTHER direction's op window, so the DVE
    chain never stalls. Interleaved independent ops measure 810 ns each.
    ACT ops are ~390 ns, so the rest is batched: e-slot scatters 2 pairs
    per op (block-reversed for bwd so strides stay positive), beta
    persists 4 pairs per op (negative-stride out). GPSIMD parks
    gamma = delta + beta into d_store in 4-step blocks.
    beta storage: t >= 256 in b3h (half store), t < 256 in a 16-slot
    rolling window (consumed within ~4 pairs by gamma parking).

  Tail: onehot = (gamma >= rowmax) via DVE reduce_max + is_ge in-place in
    d_store, in 16-step sub-chunks as soon as both directions have covered
    them; DMA out overlaps the scan. (A global V*-threshold compare was
    measured unsafe: f32 noise +-0.02 overlaps the margin distribution.)
"""

import numpy as np

B, S, D, L = 1024, 512, 128, 26
NCORES = 8
BC = B // NCORES
HALF = S // 2
NP = S - 1
NSC = 4         # scratch slots per direction
NB2 = 2         # in0 pair-slot buffers per direction (2 pairs each)
SCHUNK = 4
LA = 8          # phase-A lookahead (chunk-pairs)
TCH = 32        # tail sub-chunk (steps)
WB = 16         # b_win slots

_BUILD_CACHE = {}


def _np_vitstep(in0, in1, c0, c1, c2):
    i0 = np.asarray(in0, np.float32).reshape(in0.shape[0], -1, 27)
    T = i0[..., :26]
    e = i0[..., 26]
    i1 = np.asarray(in1, np.float32).reshape(in1.shape[0], -1, 26)[..., :26]
    s = T + i1
    r = np.maximum.accumulate(s, axis=-1)
    out = np.empty_like(i0)
    out[..., :26] = r
    out[..., 26] = r[..., 25] + e
    return out.reshape(in0.shape)


def register_vitstep():
    from concourse import dve_spec as Dv
    from concourse import dve_ops as DO
    from concourse.dve_spec import Spec, Src0, Src1, scan, AluOp
    from concourse.dve_uop import DveOpSpec, AluInp, Trigger

    for op in DO.OPS:
        if op.name == "VITSTEP_ANT":
            return op

    SRC_DONE = Trigger.SRC_TENSOR_DONE
    CNT = Trigger.COUNT
    NONE = Trigger.NONE

    def _lower_vitstep(spec, ver):
        Dv._validate_body(spec, ver)
        spec2 = Dv._hoist_stream_invariant_ops(spec)
        scans = Dv._collect(spec2.body, Dv.Scan)
        p = Dv._build_placement(spec2, scans, Dv.N_STAGES[ver], Dv.N_LANES[ver])
        base_states = list(Dv._build_state_machine(spec2, scans, [], p))
        assert len(base_states) == 2, base_states
        consume = base_states[1].consume
        assert consume == (True, True)
        sc = scans[0]
        init = Dv._scan_init(sc)
        d = p.node_stage[sc]
        reset_ov = {d: Dv._Stage(sc.op, init, sc.expr)}
        adde_ov = {
            0: Dv._Stage(AluOp.BYPASS, Src0),
            d: Dv._Stage(AluOp.ADD, AluInp.CURR_ALU_OUT, Dv.PREV),
        }
        mk = Dv._State
        states = [
            mk(placement=p, consume=consume, overrides=reset_ov,
               trigger=(SRC_DONE, CNT, NONE), next=(0, 1, 0), repeat=1),
            mk(placement=p, consume=consume,
               trigger=(SRC_DONE, CNT, NONE), next=(0, 2, 0), repeat=25),
            mk(placement=p, consume=(True, False), overrides=adde_ov,
               trigger=(SRC_DONE, CNT, NONE), next=(0, 3, 0), repeat=1),
            mk(placement=p, consume=consume, overrides=reset_ov,
               trigger=(SRC_DONE, CNT, NONE), next=(0, 1, 0), repeat=1),
        ]
        out = [Dv._assemble(s) for s in states]
        for u in out:
            u.validate(ver)
        return out

    class VitDveOp(DO.DveOp):
        def compile(self, ver):
            key = (self.name, ver)
            if (r := DO._COMPILE_CACHE.get(key)) is not None:
                return r
            result = DveOpSpec(
                name=self.name,
                opcode=DO.get_dve_sub_opcode(self.name),
                uops=_lower_vitstep(self.spec, ver),
                rd1_en=DO.has_src1(self.spec),
            )
            DO._COMPILE_CACHE[key] = result
            return result

    spec = Spec(body=scan(AluOp.MAX, Src0 + Src1), reference=_np_vitstep)
    op = VitDveOp("VITSTEP_ANT", spec, subdim=False, uops_sha={})
    DO.OPS.append(op)
    DO._SUB_OPCODE_FOR_NAME[op.name] = DO._CUSTOM_DVE_ROW_BASE + len(DO.OPS) - 1
    DO.CUSTOM_DVE_SPECS[op.name] = spec
    assert DO._SUB_OPCODE_FOR_NAME[op.name] < 0x20
    return op


def _build(s_len):
    import concourse.bass as bass
    import concourse.bacc as bacc
    import concourse.tile as tile
    import concourse.mybir as mybir

    vit = register_vitstep()
    alu = mybir.AluOpType
    f32 = mybir.dt.float32

    half = s_len // 2
    np_ = s_len - 1
    nch = s_len // SCHUNK

    nc = bacc.Bacc("TRN2", target_bir_lowering=False, debug=False)
    Xh = nc.dram_tensor("XT", (D, s_len, BC), f32, kind="ExternalInput")
    Wh = nc.dram_tensor("W", (D, L), f32, kind="ExternalInput")
    Th = nc.dram_tensor("T", (L, L), f32, kind="ExternalInput")
    Tth = nc.dram_tensor("TT", (L, L), f32, kind="ExternalInput")
    Oh = nc.dram_tensor("OUT", (BC, s_len, L), f32, kind="ExternalOutput")

    def ap_at(t, offset_elems, dims):
        a = t[:]
        return bass.AP(tensor=a.tensor, offset=a.offset + offset_elems,
                       ap=[list(a.ap[0])] + dims)

    chunk_order = []
    for c in range(nch // 2):
        chunk_order += [c, nch - 1 - c]
    if nch % 2:
        chunk_order.append(nch // 2)

    # gamma block schedule: pair -> list of (t0, t1, src) to park
    #   src: ("b3h", off) | ("bwin", slot)
    gamma_at = {}
    for k in range(259, np_ - 3, 4):          # fwd side: ts k-3..k  (256..507)
        gamma_at.setdefault(k, []).append((k - 3, k + 1, "b3h"))
    for t0 in range(4, half, 4):              # bwd side: ts t0..t0+3 (4..255)
        gamma_at.setdefault(514 - t0, []).append((t0, t0 + 4, "bwin"))
    post_gamma = [(0, 4, "bwin"), (s_len - 4, s_len - 1, "b3h")]

    # tail schedule: pieces (t0, tlen); 8-step edges go post-scan
    pieces = [(0, 8), (8, 24)] + [(t, 32) for t in range(32, s_len - 32, 32)] \
        + [(s_len - 32, 24), (s_len - 8, 8)]
    tail_at = {}
    post_tail = []
    for (t0, tl) in pieces:
        t_hi = t0 + tl - 1
        if t0 < 4 or t_hi >= s_len - 4:
            post_tail.append((t0, tl))
            continue
        r = (515 - t0) if t_hi < half else (t_hi + 4)
        if r <= np_ - 2:
            tail_at.setdefault(r, []).append((t0, tl))
        else:
            post_tail.append((t0, tl))

    with tile.TileContext(nc) as tc:
        with (
            tc.tile_pool(name="singles", bufs=1) as singles,
            tc.tile_pool(name="xstage", bufs=4) as xstage_p,
            tc.tile_pool(name="ps_e", bufs=8, space="PSUM") as ps_e,
        ):
            e_store = singles.tile([BC, s_len * L], f32)
            d_store = singles.tile([BC, s_len * L], f32)
            b3h = singles.tile([BC, half * L], f32)     # beta, t >= half
            b_win = singles.tile([BC, WB * L], f32)     # beta, t < half
            t_cat = singles.tile([BC, 2, L, L], f32)
            # in0 pair-slot bufs: [dir-buf][2 pairs][26 pages][27]
            in0f = singles.tile([BC, NB2, 2, L * 27], f32)
            in0b = singles.tile([BC, NB2, 2, L * 27], f32)
            scrf = singles.tile([BC, NSC, 704], f32)
            scrb = singles.tile([BC, NSC, 704], f32)
            c_pp = singles.tile([BC, 2, L], f32)
            gm = singles.tile([BC, TCH], f32)
            w_sb = singles.tile([D, L], f32)
            e3 = e_store.rearrange("p (s l) -> p s l", l=L)

            nc.sync.dma_start(w_sb[:], Wh[:])

            def emit_chunk(cidx):
                c0 = cidx * SCHUNK
                xs = xstage_p.tile([D, SCHUNK, BC], f32)
                nc.sync.dma_start(xs[:], Xh[:, c0:c0 + SCHUNK, :])
                ep = ps_e.tile([BC, SCHUNK, L], f32)
                for si in range(SCHUNK):
                    nc.tensor.matmul(ep[:, si, :], lhsT=xs[:, si, :],
                                     rhs=w_sb[:], start=True, stop=True)
                nc.scalar.copy(e3[:, c0:c0 + SCHUNK, :], ep[:])

            emitted = 0

            def ensure_chunks(n):
                nonlocal emitted
                while emitted < min(n, nch):
                    emit_chunk(chunk_order[emitted])
                    emitted += 1

            ensure_chunks(4)
            t_ap = Th[:]
            nc.sync.dma_start(
                t_cat[:, 1, :, :].rearrange("p a b -> p (a b)"),
                bass.AP(tensor=t_ap.tensor, offset=t_ap.offset,
                        ap=[[0, BC], [1, L * L]]),
            )
            tt_ap = Tth[:]
            nc.sync.dma_start(
                t_cat[:, 0, :, :].rearrange("p a b -> p (a b)"),
                bass.AP(tensor=tt_ap.tensor, offset=tt_ap.offset,
                        ap=[[0, BC], [1, L * L]]),
            )

            for b in range(NB2):
                for blk in range(2):
                    off = (b * 2 + blk) * (L * 27)
                    nc.vector.tensor_copy(
                        ap_at(in0f, off, [[27, L], [1, L]]), t_cat[:, 0])
                    nc.vector.tensor_copy(
                        ap_at(in0b, off, [[27, L], [1, L]]), t_cat[:, 1])

            d3 = d_store.rearrange("p (s l) -> p s l", l=L)


            # scatter e-slots for the pair group {p, p+1} (p even)
            def scatter_group(p):
                buf = (p // 2) % NB2
                if p == np_ - 1:  # last group has a single pair (510)
                    nc.scalar.copy(
                        ap_at(in0f, (buf * 2) * (L * 27) + 26, [[27, L]]),
                        e3[:, p + 1, :])
                    nc.scalar.copy(
                        ap_at(in0b, (buf * 2 + 1) * (L * 27) + 26, [[27, L]]),
                        e3[:, np_ - 1 - p, :])
                    return
                # fwd: block j holds pair p+j -> e_{p+1+j}; ascending
                nc.scalar.copy(
                    ap_at(in0f, (buf * 2) * (L * 27) + 26,
                          [[L * 27, 2], [27, L]]),
                    e3[:, p + 1:p + 3, :])
                # bwd: block j holds pair p+1-j -> block0=e_{509-p},
                # block1=e_{510-p}; in ascending {509-p, 510-p}
                nc.scalar.copy(
                    ap_at(in0b, (buf * 2) * (L * 27) + 26,
                          [[L * 27, 2], [27, L]]),
                    e3[:, np_ - 2 - p:np_ - p, :])

            # boot: d3[0] = e_0; scatter pair groups {0,1} and {2,3}
            nc.scalar.copy(d3[:, 0, :], e3[:, 0, :])
            scatter_group(0)
            scatter_group(2)

            def emit_tail(piece):
                c0, tl = piece
                gsrc = d3[:, c0:c0 + tl, :]
                nc.vector.reduce_max(gm[:, :tl], gsrc,
                                     axis=mybir.AxisListType.X)
                gm_bc = (gm[:, :tl].rearrange("p (t o) -> p t o", o=1)
                         .broadcast_to((BC, tl, L)))
                nc.vector.tensor_tensor(gsrc, gsrc, gm_bc, op=alu.is_ge)
                nc.sync.dma_start(
                    Oh[:, c0:c0 + tl, :].rearrange("p s l -> p (s l)"),
                    d_store[:, c0 * L:(c0 + tl) * L])

            def bpersist_to(bt, n, sf0):
                """ACT: b[bt], b[bt-1], .., b[bt-n+1] <- scrb slots sf0..sf0+n-1
                col 25 (bt descending as slot ascends)."""
                src = ap_at(scrb, sf0 * 704 + 25, [[704, n], [27, L]]) if n > 1 \
                    else ap_at(scrb, sf0 * 704 + 25, [[27, L]])
                if bt >= half:
                    assert bt - n + 1 >= half
                    dst = ap_at(b3h, (bt - half) * L, [[-L, n], [1, L]]) \
                        if n > 1 else ap_at(b3h, (bt - half) * L, [[1, L]])
                    nc.scalar.copy(dst, src)
                else:
                    # b_win slots bt%WB descending; split at window wrap
                    done = 0
                    while done < n:
                        b0 = bt - done
                        run = min(n - done, b0 % WB + 1)
                        s = ap_at(scrb, (sf0 + done) * 704 + 25,
                                  [[704, run], [27, L]]) if run > 1 else \
                            ap_at(scrb, (sf0 + done) * 704 + 25, [[27, L]])
                        dd = ap_at(b_win, (b0 % WB) * L, [[-L, run], [1, L]]) \
                            if run > 1 else ap_at(b_win, (b0 % WB) * L, [[1, L]])
                        nc.scalar.copy(dd, s)
                        done += run

            # ---- scan pairs ----
            for k in range(np_):
                ft, bt = k + 1, s_len - 2 - k
                buf, blk, sf = (k // 2) % NB2, k % 2, k % NSC
                # DVE ops
                if k == 0:
                    in1f = ap_at(e_store, 0, [[0, L], [1, L]])
                    in1b = ap_at(e_store, (s_len - 1) * L, [[0, L], [1, L]])
                else:
                    in1f = ap_at(d_store, k * L, [[0, L], [1, L]])
                    in1b = ap_at(c_pp, ((k - 1) % 2) * L, [[0, L], [1, L]])
                nc.vector._custom_dve(
                    vit, out=ap_at(scrf, sf * 704, [[27, L], [1, 27]]),
                    in0=ap_at(in0f, (buf * 2 + blk) * (L * 27),
                              [[27, L], [1, 27]]),
                    in1=in1f)
                nc.vector._custom_dve(
                    vit, out=ap_at(scrb, sf * 704, [[27, L], [1, 27]]),
                    in0=ap_at(in0b, (buf * 2 + (1 - blk)) * (L * 27),
                              [[27, L], [1, 27]]),
                    in1=in1b)
                # ACT chain persists (each hidden under the other dir's op)
                nc.scalar.copy(d3[:, ft, :], ap_at(scrf, sf * 704 + 26,
                                                   [[27, L]]))
                nc.scalar.copy(ap_at(c_pp, (k % 2) * L, [[1, L]]),
                               ap_at(scrb, sf * 704 + 26, [[27, L]]))
                # batched beta persist: at k%4==3, pairs k-3..k (slots 0..3)
                if k % 4 == 3:
                    if bt + 3 >= half and bt < half:
                        nh = bt + 4 - half      # in b3h
                        bpersist_to(bt + 3, nh, 0)
                        bpersist_to(bt + 3 - nh, 4 - nh, nh)
                    else:
                        bpersist_to(bt + 3, 4, 0)
                elif k >= np_ - 3:              # tail pairs 508..510: singles
                    bpersist_to(bt, 1, sf)
                # scatters for pair group {k+3, k+4} at odd k
                if k % 2 == 1 and k + 3 <= np_ - 1:
                    scatter_group(k + 3)
                # GPSIMD gamma parking
                for (t0, t1, src) in gamma_at.get(k, ()):
                    if src == "b3h":
                        bsrc = ap_at(b3h, (t0 - half) * L, [[1, (t1 - t0) * L]])
                    else:
                        bsrc = ap_at(b_win, (t0 % WB) * L, [[1, (t1 - t0) * L]])
                    nc.gpsimd.tensor_tensor(
                        d_store[:, t0 * L:t1 * L],
                        d_store[:, t0 * L:t1 * L], bsrc, op=alu.add)
                for c in tail_at.get(k, ()):
                    emit_tail(c)
                if k % 2 == 0:
                    ensure_chunks(min(4 + k, k // 2 + 24))

            for (t0, t1, src) in post_gamma:
                if src == "b3h":
                    bsrc = ap_at(b3h, (t0 - half) * L, [[1, (t1 - t0) * L]])
                else:
                    bsrc = ap_at(b_win, (t0 % WB) * L, [[1, (t1 - t0) * L]])
                nc.gpsimd.tensor_tensor(
                    d_store[:, t0 * L:t1 * L],
                    d_store[:, t0 * L:t1 * L], bsrc, op=alu.add)
            for c in post_tail:
                emit_tail(c)

    nc.compile()
    return nc


def _get(s_len):
    if s_len not in _BUILD_CACHE:
        _BUILD_CACHE[s_len] = _build(s_len)
    return _BUILD_CACHE[s_len]


LAST_RESULT = None


def kernel(X, W, T):
    global LAST_RESULT
    from concourse.bass_utils import run_bass_kernel_spmd

    X = np.ascontiguousarray(X, dtype=np.float32)
    W = np.ascontiguousarray(W, dtype=np.float32)
    T = np.ascontiguousarray(T, dtype=np.float32)
    s_len = X.shape[1]
    nc = _get(s_len)
    in_maps = []
    for c in range(NCORES):
        xt = np.ascontiguousarray(
            X[c * BC:(c + 1) * BC].transpose(2, 1, 0))  # [D, S, BC]
        in_maps.append({"XT": xt, "W": W, "T": T,
                        "TT": np.ascontiguousarray(T.T)})
    res = run_bass_kernel_spmd(nc, in_maps, core_ids=list(range(NCORES)))
    LAST_RESULT = res
    return np.concatenate([r["OUT"] for r in res.results], axis=0)
